# revision 17
# baseline (speedup 1.0000x reference)
"""Trainium2 Bass kernel for nn_Attention_86638080295542.

Multi-head attention (12 heads, d=64) with the reference's v=k quirk:
    q = x @ Wq.T + bq ; k = x @ Wk.T + bk ; v = k
    out = softmax(q k^T / sqrt(d)) @ v ;  y = out @ Wo.T + bo

Sharding: batch (B=8) data-parallel across the 8 NeuronCores — core c
computes batch element c end-to-end, no collectives.

Per-core dataflow (all "T" tensors keep the contraction dim on SBUF
partitions so every matmul is a natural lhsT.T @ rhs):
  xT[e,s], WqT/WkT/WoT[e_in,e_out] are pre-transposed on the host.
  qT = Wq @ xT   (+bq per-partition)        [768,1024]
  kT = Wk @ xT   (+bk per-partition)        [768,1024]
  vaug[j, jb, h, 0:64] = k natural (PE transpose of kT), col 64 = 1.0
  per head h: pT[j,i] = exp(scale * kT_h^T qT_h)  (no max-subtraction:
     logits are O(1) for this problem, softmax is shift-invariant)
  outT_h[d,i] (+ rowsum in row 64) = vaug^T @ pT, accumulated over j
  normalize: outT_h *= 1/rowsum (broadcast via ones-matmul on PE)
  y = outT^T @ WoT + bo
"""

from contextlib import ExitStack

import numpy as np

import concourse.bass as bass
import concourse.tile as tile
from concourse import bacc, mybir
from concourse import bass_utils

S = 1024          # sequence length
E = 768           # embed dim
H = 12            # heads
DH = 64           # head dim
P = 128           # partitions
KT = E // P       # 6 k-tiles over embed dim
ST = S // P       # 8 tiles over sequence
NCH = S // 512    # 2 free-dim chunks of 512 over sequence
SCALE = DH ** -0.5
NCORES = 8

F32 = mybir.dt.float32
F32R = mybir.dt.float32r
BF16 = mybir.dt.bfloat16


def _emit(nc, tc, ctx, iters=1):
    xT_d = nc.dram_tensor("xT", [E, S], F32R, kind="ExternalInput")
    WqT_d = nc.dram_tensor("WqT", [E, E], F32R, kind="ExternalInput")
    WkT_d = nc.dram_tensor("WkT", [E, E], F32R, kind="ExternalInput")
    WoT_d = nc.dram_tensor("WoT", [E, E], F32R, kind="ExternalInput")
    bq_d = nc.dram_tensor("bq", [E], F32, kind="ExternalInput")
    bk_d = nc.dram_tensor("bk", [E], F32, kind="ExternalInput")
    bo_d = nc.dram_tensor("bo", [E], F32, kind="ExternalInput")
    y_d = nc.dram_tensor("y", [S, E], F32, kind="ExternalOutput")

    Exp = mybir.ActivationFunctionType.Exp

    const = ctx.enter_context(tc.tile_pool(name="const", bufs=1))
    xt_pool = ctx.enter_context(tc.tile_pool(name="xt", bufs=1))
    outt_pool = ctx.enter_context(tc.tile_pool(name="outt", bufs=1))
    w_pool = ctx.enter_context(tc.tile_pool(name="w", bufs=2))
    wo_pool = ctx.enter_context(tc.tile_pool(name="wo", bufs=1))
    vaug_pool = ctx.enter_context(tc.tile_pool(name="vaug", bufs=1))
    qt_pool = ctx.enter_context(tc.tile_pool(name="qt", bufs=3))
    kt_pool = ctx.enter_context(tc.tile_pool(name="kt", bufs=3))
    pt_pool = ctx.enter_context(tc.tile_pool(name="pt", bufs=8))
    ysb_pool = ctx.enter_context(tc.tile_pool(name="ysb", bufs=2))
    pvsb_pool = ctx.enter_context(tc.tile_pool(name="pvsb", bufs=4))
    rc_pool = ctx.enter_context(tc.tile_pool(name="rc", bufs=2))
    ps_s = ctx.enter_context(tc.tile_pool(name="ps_s", bufs=3, space="PSUM"))
    ps_pv = ctx.enter_context(tc.tile_pool(name="ps_pv", bufs=1, space="PSUM"))

    if iters > 1:
        ctx.enter_context(tc.For_i(0, iters, 1))

    # ---- constants ----
    # gpsimd/memset can't emit float32r, so build fp32 then copy-round on DVE
    # (0.0/1.0 are exactly representable, so the copy is exact).
    ident_f32 = const.tile([P, P], F32, tag="ident_f32")
    from concourse.masks import make_identity
    make_identity(nc, ident_f32[:])
    identity = const.tile([P, P], F32R, tag="ident")
    nc.vector.tensor_copy(identity[:], ident_f32[:])
    ones64_f32 = const.tile([1, DH], F32, tag="ones64_f32")
    nc.vector.memset(ones64_f32[:], 1.0)
    ones64 = const.tile([1, DH], F32R, tag="ones64")
    nc.vector.tensor_copy(ones64[:], ones64_f32[:])
    bq_sb = const.tile([P, KT], F32, tag="bq")
    nc.sync.dma_start(bq_sb[:], bq_d.ap().rearrange("(t p) -> p t", p=P))
    bk_sb = const.tile([P, KT], F32, tag="bk")
    nc.sync.dma_start(bk_sb[:], bk_d.ap().rearrange("(t p) -> p t", p=P))
    # bo broadcast to all 128 partitions via a 0-step partition AP (DRAM APs
    # are not partitioned, so a 0-step leading dim is legal here)
    bo_bc = const.tile([P, E], F32, tag="bo")
    bo_ap = bo_d.ap()
    bo_bcast_src = bass.AP(bo_ap.tensor, bo_ap.offset, [[0, P], [1, E]])
    nc.sync.dma_start(bo_bc[:], bo_bcast_src)

    # ---- input loads (per k-tile so compute can start early) ----
    xT_sb = xt_pool.tile([P, KT, S], F32R, tag="xt")
    WqT_sb = w_pool.tile([P, KT, E], F32R, tag="w")
    WkT_sb = w_pool.tile([P, KT, E], F32R, tag="w")
    WoT_sb = wo_pool.tile([P, KT, E], F32R, tag="wo")
    xT_r = xT_d.ap().rearrange("(t p) s -> p t s", p=P)
    WqT_r = WqT_d.ap().rearrange("(t p) e -> p t e", p=P)
    WkT_r = WkT_d.ap().rearrange("(t p) e -> p t e", p=P)
    WoT_r = WoT_d.ap().rearrange("(t p) e -> p t e", p=P)
    for t in range(KT):
        nc.sync.dma_start(xT_sb[:, t, :], xT_r[:, t, :])
        nc.sync.dma_start(WqT_sb[:, t, :], WqT_r[:, t, :])
        nc.sync.dma_start(WkT_sb[:, t, :], WkT_r[:, t, :])
        nc.sync.dma_start(WoT_sb[:, t, :], WoT_r[:, t, :])

    vaug = vaug_pool.tile([P, ST, H, DH + 1], BF16, tag="vaug")
    for jb in range(ST):
        nc.vector.memset(vaug[:, jb, :, DH:DH + 1], 1.0)
    outT_sb = outt_pool.tile([P, KT, S], F32R, tag="outt")

    # ---- per head-pair: projections (tq=hp), vaug transposes (t=hp),
    # then the pair's attention. Interleaving lets ACT exp work start while
    # the PE is still projecting later tiles, overlapping the two engines.
    for hp in range(KT):
        # projections for e-tile hp: qT/kT rows [128*hp, 128*hp+128), written
        # into per-pair rotating tiles (only this pair ever reads them)
        qp = qt_pool.tile([P, S], F32R, tag="qt", name=f"qp_{hp}")
        kp = kt_pool.tile([P, S], F32R, tag="kt", name=f"kp_{hp}")
        for W_sb, b_sb, out_sb in ((WqT_sb, bq_sb, qp), (WkT_sb, bk_sb, kp)):
            for c in range(NCH):
                ps = ps_s.tile([P, 512], F32, tag="ps_s")
                for t in range(KT):
                    nc.tensor.matmul(
                        ps[:],
                        W_sb[:, t, 128 * hp:128 * hp + 128],
                        xT_sb[:, t, 512 * c:512 * c + 512],
                        start=(t == 0), stop=(t == KT - 1),
                    )
                nc.vector.tensor_scalar_add(
                    out_sb[:, 512 * c:512 * c + 512], ps[:], b_sb[:, hp:hp + 1]
                )
        # vaug slices for heads 2hp, 2hp+1 via PE transposes of kT tile hp
        for g in range(2):
            ps = ps_s.tile([P, 512], F32R, tag="ps_s")
            for j4 in range(4):
                jb = 4 * g + j4
                nc.tensor.transpose(
                    ps[:, 128 * j4:128 * j4 + 128],
                    kp[:, 128 * jb:128 * jb + 128],
                    identity[:],
                )
            nc.vector.tensor_copy(
                vaug[:, 4 * g:4 * g + 4, 2 * hp:2 * hp + 2, 0:DH],
                ps[:].rearrange("p (a b c) -> p a b c", a=4, b=2, c=DH),
            )
        # attention for the two heads of this pair, one head at a time.
        # Score psums are triple-buffered [128, S] tiles so the PE can run
        # a couple of j-blocks ahead of the ACT exp evictions.
        for h in (2 * hp, 2 * hp + 1):
            po = DH * (h % 2)
            pv = ps_pv.tile([DH + 1, S], F32, tag="ps_pv", name=f"pv_{h}")
            for jb in range(ST):
                sps = ps_s.tile([P, S], F32, tag="ps_s", name=f"sps_{h}_{jb}")
                for c in range(NCH):
                    nc.tensor.matmul(
                        sps[:, 512 * c:512 * c + 512],
                        kp[po:po + DH, 128 * jb:128 * jb + 128],
                        qp[po:po + DH, 512 * c:512 * c + 512],
                        start=True, stop=True,
                    )
                pt = pt_pool.tile([P, S], BF16, tag="pt")
                nc.scalar.activation(pt[:], sps[:], Exp, scale=SCALE)
                for c in range(NCH):
                    nc.tensor.matmul(
                        pv[:, 512 * c:512 * c + 512],
                        vaug[:, jb, h, :],
                        pt[:, 512 * c:512 * c + 512],
                        start=(jb == 0), stop=(jb == ST - 1),
                    )
            # evict pv to SBUF right away (frees the PSUM bank), then
            # normalize: reciprocal of rowsum row, rank-1 PE broadcast into a
            # recycled ps_s slot, multiply straight from PSUM into outT.
            pvsb = pvsb_pool.tile([DH + 1, S], F32, tag="pvsb", name=f"pvsb_{h}")
            nc.vector.tensor_copy(pvsb[:], pv[:])
            rc = rc_pool.tile([1, S], F32R, tag="rc", name=f"rc_{h}")
            with nc.allow_low_precision(reason="tf32 reciprocal of softmax denominator"):
                nc.vector.reciprocal(rc[:], pvsb[DH:DH + 1, :])
            rb = ps_s.tile([P, S], F32, tag="ps_s", name=f"rb_{h}")
            for c in range(NCH):
                nc.tensor.matmul(rb[0:DH, 512 * c:512 * c + 512], ones64[:],
                                 rc[:, 512 * c:512 * c + 512], start=True, stop=True)
            nc.vector.tensor_mul(
                outT_sb[po:po + DH, hp, :], pvsb[0:DH, :], rb[0:DH, :],
            )

    # ---- output projection: y = outT^T @ WoT + bo ----
    y_r = y_d.ap().rearrange("(st p) e -> st p e", p=P)
    for st in range(ST):
        ysb = ysb_pool.tile([P, E], F32, tag="ysb")
        for n0 in (0, 384):
            yps = ps_s.tile([P, 512], F32, tag="ps_s")
            for t in range(KT):
                nc.tensor.matmul(
                    yps[:, 0:384],
                    outT_sb[:, t, 128 * st:128 * st + 128],
                    WoT_sb[:, t, n0:n0 + 384],
                    start=(t == 0), stop=(t == KT - 1),
                )
            nc.vector.tensor_add(ysb[:, n0:n0 + 384], yps[:, 0:384], bo_bc[:, n0:n0 + 384])
        nc.sync.dma_start(y_r[st], ysb[:])


_NC_CACHE = {}


def build(iters=1):
    nc = _NC_CACHE.get(iters)
    if nc is None:
        nc = bacc.Bacc("TRN2", target_bir_lowering=False, debug=False)
        with tile.TileContext(nc) as tc, ExitStack() as ctx:
            _emit(nc, tc, ctx, iters=iters)
        nc.compile()
        _NC_CACHE[iters] = nc
    return nc


def _round_tf32(a):
    """Round fp32 to tf32 (10 explicit mantissa bits), RNE, fp32 container."""
    a = np.ascontiguousarray(np.asarray(a, dtype=np.float32))
    u = a.view(np.uint32)
    lsb = (u >> np.uint32(13)) & np.uint32(1)
    r = (u + np.uint32(0x0FFF) + lsb) & np.uint32(0xFFFFE000)
    return r.view(np.float32)


def make_in_maps(x, Wq, bq, Wk, bk, Wo, bo):
    WqT = _round_tf32(np.asarray(Wq, dtype=np.float32).T)
    WkT = _round_tf32(np.asarray(Wk, dtype=np.float32).T)
    WoT = _round_tf32(np.asarray(Wo, dtype=np.float32).T)
    bq = np.ascontiguousarray(np.asarray(bq, dtype=np.float32))
    bk = np.ascontiguousarray(np.asarray(bk, dtype=np.float32))
    bo = np.ascontiguousarray(np.asarray(bo, dtype=np.float32))
    x = np.asarray(x, dtype=np.float32)
    return [
        {
            "xT": _round_tf32(x[c].T),
            "WqT": WqT, "WkT": WkT, "WoT": WoT,
            "bq": bq, "bk": bk, "bo": bo,
        }
        for c in range(NCORES)
    ]


def kernel(x, Wq, bq, Wk, bk, Wo, bo):
    nc = build()
    in_maps = make_in_maps(x, Wq, bq, Wk, bk, Wo, bo)
    res = bass_utils.run_bass_kernel_spmd(nc, in_maps, core_ids=list(range(NCORES)))
    return np.stack([res.results[c]["y"] for c in range(NCORES)]).astype(np.float32)


# revision 18
# speedup vs baseline: 1.0177x; 1.0177x over previous
"""Trainium2 Bass kernel for nn_Attention_86638080295542.

Multi-head attention (12 heads, d=64) with the reference's v=k quirk:
    q = x @ Wq.T + bq ; k = x @ Wk.T + bk ; v = k
    out = softmax(q k^T / sqrt(d)) @ v ;  y = out @ Wo.T + bo

Sharding: batch (B=8) data-parallel across the 8 NeuronCores — core c
computes batch element c end-to-end, no collectives.

Per-core dataflow (all "T" tensors keep the contraction dim on SBUF
partitions so every matmul is a natural lhsT.T @ rhs):
  xT[e,s], WqT/WkT/WoT[e_in,e_out] are pre-transposed on the host.
  qT = Wq @ xT   (+bq per-partition)        [768,1024]
  kT = Wk @ xT   (+bk per-partition)        [768,1024]
  vaug[j, jb, h, 0:64] = k natural (PE transpose of kT), col 64 = 1.0
  per head h: pT[j,i] = exp(scale * kT_h^T qT_h)  (no max-subtraction:
     logits are O(1) for this problem, softmax is shift-invariant)
  outT_h[d,i] (+ rowsum in row 64) = vaug^T @ pT, accumulated over j
  normalize: outT_h *= 1/rowsum (broadcast via ones-matmul on PE)
  y = outT^T @ WoT + bo
"""

from contextlib import ExitStack

import numpy as np

import concourse.bass as bass
import concourse.tile as tile
from concourse import bacc, mybir
from concourse import bass_utils

S = 1024          # sequence length
E = 768           # embed dim
H = 12            # heads
DH = 64           # head dim
P = 128           # partitions
KT = E // P       # 6 k-tiles over embed dim
ST = S // P       # 8 tiles over sequence
NCH = S // 512    # 2 free-dim chunks of 512 over sequence
SCALE = DH ** -0.5
NCORES = 8

F32 = mybir.dt.float32
F32R = mybir.dt.float32r
BF16 = mybir.dt.bfloat16


def _emit(nc, tc, ctx, iters=1):
    xT_d = nc.dram_tensor("xT", [E, S], F32R, kind="ExternalInput")
    WqT_d = nc.dram_tensor("WqT", [E, E], F32R, kind="ExternalInput")
    WkT_d = nc.dram_tensor("WkT", [E, E], F32R, kind="ExternalInput")
    WoT_d = nc.dram_tensor("WoT", [E, E], F32R, kind="ExternalInput")
    bq_d = nc.dram_tensor("bq", [E], F32, kind="ExternalInput")
    bk_d = nc.dram_tensor("bk", [E], F32, kind="ExternalInput")
    bo_d = nc.dram_tensor("bo", [E], F32, kind="ExternalInput")
    y_d = nc.dram_tensor("y", [S, E], F32, kind="ExternalOutput")

    Exp = mybir.ActivationFunctionType.Exp

    const = ctx.enter_context(tc.tile_pool(name="const", bufs=1))
    xt_pool = ctx.enter_context(tc.tile_pool(name="xt", bufs=1))
    outt_pool = ctx.enter_context(tc.tile_pool(name="outt", bufs=1))
    w_pool = ctx.enter_context(tc.tile_pool(name="w", bufs=2))
    wo_pool = ctx.enter_context(tc.tile_pool(name="wo", bufs=1))
    vaug_pool = ctx.enter_context(tc.tile_pool(name="vaug", bufs=1))
    qt_pool = ctx.enter_context(tc.tile_pool(name="qt", bufs=3))
    kt_pool = ctx.enter_context(tc.tile_pool(name="kt", bufs=3))
    pt_pool = ctx.enter_context(tc.tile_pool(name="pt", bufs=8))
    ysb_pool = ctx.enter_context(tc.tile_pool(name="ysb", bufs=2))
    pvsb_pool = ctx.enter_context(tc.tile_pool(name="pvsb", bufs=4))
    rc_pool = ctx.enter_context(tc.tile_pool(name="rc", bufs=2))
    ps_s = ctx.enter_context(tc.tile_pool(name="ps_s", bufs=3, space="PSUM"))
    ps_pv = ctx.enter_context(tc.tile_pool(name="ps_pv", bufs=1, space="PSUM"))

    if iters > 1:
        ctx.enter_context(tc.For_i(0, iters, 1))

    # ---- constants ----
    # gpsimd/memset can't emit float32r, so build fp32 then copy-round on DVE
    # (0.0/1.0 are exactly representable, so the copy is exact).
    ident_f32 = const.tile([P, P], F32, tag="ident_f32")
    from concourse.masks import make_identity
    make_identity(nc, ident_f32[:])
    identity = const.tile([P, P], F32R, tag="ident")
    nc.vector.tensor_copy(identity[:], ident_f32[:])
    ones64_f32 = const.tile([1, DH], F32, tag="ones64_f32")
    nc.vector.memset(ones64_f32[:], 1.0)
    ones64 = const.tile([1, DH], F32R, tag="ones64")
    nc.vector.tensor_copy(ones64[:], ones64_f32[:])
    bq_sb = const.tile([P, KT], F32, tag="bq")
    nc.sync.dma_start(bq_sb[:], bq_d.ap().rearrange("(t p) -> p t", p=P))
    bk_sb = const.tile([P, KT], F32, tag="bk")
    nc.sync.dma_start(bk_sb[:], bk_d.ap().rearrange("(t p) -> p t", p=P))
    # bo broadcast to all 128 partitions via a 0-step partition AP (DRAM APs
    # are not partitioned, so a 0-step leading dim is legal here)
    bo_bc = const.tile([P, E], F32, tag="bo")
    bo_ap = bo_d.ap()
    bo_bcast_src = bass.AP(bo_ap.tensor, bo_ap.offset, [[0, P], [1, E]])
    nc.sync.dma_start(bo_bc[:], bo_bcast_src)

    # ---- input loads (per k-tile so compute can start early) ----
    xT_sb = xt_pool.tile([P, KT, S], F32R, tag="xt")
    WqT_sb = w_pool.tile([P, KT, E], F32R, tag="w")
    WkT_sb = w_pool.tile([P, KT, E], F32R, tag="w")
    WoT_sb = wo_pool.tile([P, KT, E], F32R, tag="wo")
    xT_r = xT_d.ap().rearrange("(t p) s -> p t s", p=P)
    WqT_r = WqT_d.ap().rearrange("(t p) e -> p t e", p=P)
    WkT_r = WkT_d.ap().rearrange("(t p) e -> p t e", p=P)
    WoT_r = WoT_d.ap().rearrange("(t p) e -> p t e", p=P)
    for t in range(KT):
        nc.sync.dma_start(xT_sb[:, t, :], xT_r[:, t, :])
        nc.sync.dma_start(WqT_sb[:, t, :], WqT_r[:, t, :])
        nc.sync.dma_start(WkT_sb[:, t, :], WkT_r[:, t, :])
        nc.sync.dma_start(WoT_sb[:, t, :], WoT_r[:, t, :])

    vaug = vaug_pool.tile([P, ST, H, DH + 1], BF16, tag="vaug")
    for jb in range(ST):
        nc.vector.memset(vaug[:, jb, :, DH:DH + 1], 1.0)
    outT_sb = outt_pool.tile([P, KT, S], F32R, tag="outt")

    # ---- per head-pair: projections (tq=hp), vaug transposes (t=hp),
    # then the pair's attention. Interleaving lets ACT exp work start while
    # the PE is still projecting later tiles, overlapping the two engines.
    for hp in range(KT):
        # projections for e-tile hp: qT/kT rows [128*hp, 128*hp+128), written
        # into per-pair rotating tiles (only this pair ever reads them)
        qp = qt_pool.tile([P, S], F32R, tag="qt", name=f"qp_{hp}")
        kp = kt_pool.tile([P, S], F32R, tag="kt", name=f"kp_{hp}")
        for W_sb, b_sb, out_sb in ((WqT_sb, bq_sb, qp), (WkT_sb, bk_sb, kp)):
            for c in range(NCH):
                ps = ps_s.tile([P, 512], F32, tag="ps_s")
                for t in range(KT):
                    nc.tensor.matmul(
                        ps[:],
                        W_sb[:, t, 128 * hp:128 * hp + 128],
                        xT_sb[:, t, 512 * c:512 * c + 512],
                        start=(t == 0), stop=(t == KT - 1),
                    )
                nc.vector.tensor_scalar_add(
                    out_sb[:, 512 * c:512 * c + 512], ps[:], b_sb[:, hp:hp + 1]
                )
        # vaug slices for heads 2hp, 2hp+1 via PE transposes of kT tile hp
        for g in range(2):
            ps = ps_s.tile([P, 512], F32R, tag="ps_s")
            for j4 in range(4):
                jb = 4 * g + j4
                nc.tensor.transpose(
                    ps[:, 128 * j4:128 * j4 + 128],
                    kp[:, 128 * jb:128 * jb + 128],
                    identity[:],
                )
            nc.vector.tensor_copy(
                vaug[:, 4 * g:4 * g + 4, 2 * hp:2 * hp + 2, 0:DH],
                ps[:].rearrange("p (a b c) -> p a b c", a=4, b=2, c=DH),
            )
        # attention for the two heads of this pair, one head at a time.
        # Score psums are triple-buffered [128, S] tiles so the PE can run
        # a couple of j-blocks ahead of the ACT exp evictions.
        for h in (2 * hp, 2 * hp + 1):
            po = DH * (h % 2)
            pv = ps_pv.tile([DH + 1, S], F32, tag="ps_pv", name=f"pv_{h}")

            def pv_mms(jb, pt):
                for c in range(NCH):
                    nc.tensor.matmul(
                        pv[:, 512 * c:512 * c + 512],
                        vaug[:, jb, h, :],
                        pt[:, 512 * c:512 * c + 512],
                        start=(jb == 0), stop=(jb == ST - 1),
                    )

            # software-pipelined by one j-block: the PE issues scores(jb)
            # before PV(jb-1), so exp(jb-1) on ACT overlaps scores(jb) on PE
            # instead of stalling the PE.
            prev = None
            for jb in range(ST):
                sps = ps_s.tile([P, S], F32, tag="ps_s", name=f"sps_{h}_{jb}")
                for c in range(NCH):
                    nc.tensor.matmul(
                        sps[:, 512 * c:512 * c + 512],
                        kp[po:po + DH, 128 * jb:128 * jb + 128],
                        qp[po:po + DH, 512 * c:512 * c + 512],
                        start=True, stop=True,
                    )
                pt = pt_pool.tile([P, S], BF16, tag="pt")
                nc.scalar.activation(pt[:], sps[:], Exp, scale=SCALE)
                if prev is not None:
                    pv_mms(jb - 1, prev)
                prev = pt
            pv_mms(ST - 1, prev)
            # evict pv to SBUF right away (frees the PSUM bank), then
            # normalize: reciprocal of rowsum row, rank-1 PE broadcast into a
            # recycled ps_s slot, multiply straight from PSUM into outT.
            pvsb = pvsb_pool.tile([DH + 1, S], F32, tag="pvsb", name=f"pvsb_{h}")
            nc.vector.tensor_copy(pvsb[:], pv[:])
            rc = rc_pool.tile([1, S], F32R, tag="rc", name=f"rc_{h}")
            with nc.allow_low_precision(reason="tf32 reciprocal of softmax denominator"):
                nc.vector.reciprocal(rc[:], pvsb[DH:DH + 1, :])
            rb = ps_s.tile([P, S], F32, tag="ps_s", name=f"rb_{h}")
            for c in range(NCH):
                nc.tensor.matmul(rb[0:DH, 512 * c:512 * c + 512], ones64[:],
                                 rc[:, 512 * c:512 * c + 512], start=True, stop=True)
            nc.vector.tensor_mul(
                outT_sb[po:po + DH, hp, :], pvsb[0:DH, :], rb[0:DH, :],
            )

    # ---- output projection: y = outT^T @ WoT + bo ----
    y_r = y_d.ap().rearrange("(st p) e -> st p e", p=P)
    for st in range(ST):
        ysb = ysb_pool.tile([P, E], F32, tag="ysb")
        for n0 in (0, 384):
            yps = ps_s.tile([P, 512], F32, tag="ps_s")
            for t in range(KT):
                nc.tensor.matmul(
                    yps[:, 0:384],
                    outT_sb[:, t, 128 * st:128 * st + 128],
                    WoT_sb[:, t, n0:n0 + 384],
                    start=(t == 0), stop=(t == KT - 1),
                )
            nc.vector.tensor_add(ysb[:, n0:n0 + 384], yps[:, 0:384], bo_bc[:, n0:n0 + 384])
        nc.sync.dma_start(y_r[st], ysb[:])


_NC_CACHE = {}


def build(iters=1):
    nc = _NC_CACHE.get(iters)
    if nc is None:
        nc = bacc.Bacc("TRN2", target_bir_lowering=False, debug=False)
        with tile.TileContext(nc) as tc, ExitStack() as ctx:
            _emit(nc, tc, ctx, iters=iters)
        nc.compile()
        _NC_CACHE[iters] = nc
    return nc


def _round_tf32(a):
    """Round fp32 to tf32 (10 explicit mantissa bits), RNE, fp32 container."""
    a = np.ascontiguousarray(np.asarray(a, dtype=np.float32))
    u = a.view(np.uint32)
    lsb = (u >> np.uint32(13)) & np.uint32(1)
    r = (u + np.uint32(0x0FFF) + lsb) & np.uint32(0xFFFFE000)
    return r.view(np.float32)


def make_in_maps(x, Wq, bq, Wk, bk, Wo, bo):
    WqT = _round_tf32(np.asarray(Wq, dtype=np.float32).T)
    WkT = _round_tf32(np.asarray(Wk, dtype=np.float32).T)
    WoT = _round_tf32(np.asarray(Wo, dtype=np.float32).T)
    bq = np.ascontiguousarray(np.asarray(bq, dtype=np.float32))
    bk = np.ascontiguousarray(np.asarray(bk, dtype=np.float32))
    bo = np.ascontiguousarray(np.asarray(bo, dtype=np.float32))
    x = np.asarray(x, dtype=np.float32)
    return [
        {
            "xT": _round_tf32(x[c].T),
            "WqT": WqT, "WkT": WkT, "WoT": WoT,
            "bq": bq, "bk": bk, "bo": bo,
        }
        for c in range(NCORES)
    ]


def kernel(x, Wq, bq, Wk, bk, Wo, bo):
    nc = build()
    in_maps = make_in_maps(x, Wq, bq, Wk, bk, Wo, bo)
    res = bass_utils.run_bass_kernel_spmd(nc, in_maps, core_ids=list(range(NCORES)))
    return np.stack([res.results[c]["y"] for c in range(NCORES)]).astype(np.float32)
